# revision 14
# baseline (speedup 1.0000x reference)
"""Trainium2 Bass kernel for nn_GCN_13657996001664 (2-layer GCN + BN/ReLU + RVQ + heads).

Strategy (8 NeuronCores, node-parallel):
  - Each core owns 12500 nodes (padded to 12544 = 98*128).
  - Per GCN layer: h = x @ W on own nodes; g = h * dinv written to a local DRAM
    block; AllGather replicates the full g table (100352 rows) on every core.
  - Edge aggregation: edges bucketed by (dst tile, src-table chunk); messages
    fetched with dma_gather (int16 indices, 4 chunks of 32768 rows); per
    128-edge column a one-hot selection matrix (built on DVE from a dst-lane
    vector) routes messages into a per-tile PSUM accumulator via the
    TensorEngine: x^T[hid, dst] += msg^T @ Sel.
  - BatchNorm via AllReduce of per-channel partial sums; RVQ argmin via
    s = 2 r.c - |c|^2 and max_with_indices; q via one-hot matmul.
All floating-point math runs on device; the host only shards, reorders
integer index structures, and reassembles outputs.
"""

import numpy as np

N, E, IN_C, HID, OUT_C = 100000, 1600000, 128, 128, 40
NUM_CODES, GROUPS, NUM_LAYERS = 32, 3, 2
BN_EPS = 1e-5
NCORES = 8
PC = N // NCORES              # 12500 own nodes
PCP = 12544                   # padded (98 tiles of 128)
NT = PCP // 128               # 98
R = NCORES * PCP              # 100352 table rows
CHUNK = 32768
NCHUNK = 4
BT = 4                        # tiles per psum quad / gather group
MAXJ = 8                      # max 128-edge columns per gather instruction
ZROW = {0: 12500, 1: 37588, 2: 75220, 3: 100308}   # a zero (pad) row inside each chunk


# ----------------------------------------------------------------- host prep

def _prep_edges(edge_index):
    row = edge_index[0].astype(np.int64)
    col = edge_index[1].astype(np.int64)
    loop = np.arange(N, dtype=np.int64)
    r_all = np.concatenate([row, loop])
    c_all = np.concatenate([col, loop])
    deg = np.bincount(r_all, minlength=N)          # includes self-loop

    owner = c_all // PC
    slot = owner * PCP + (c_all - owner * PC)
    chunk = slot // CHUNK
    sloc = slot - chunk * CHUNK

    dst_core = r_all // PC
    dst_local = r_all - dst_core * PC
    tile = dst_local // 128
    lane = dst_local % 128

    counts = np.zeros((NCORES, NT, NCHUNK), np.int64)
    np.add.at(counts, (dst_core, tile, chunk), 1)
    Jtc = (counts.max(axis=0) + 127) // 128        # [NT, NCHUNK] cols, shared by all cores
    return dict(deg=deg, dst_core=dst_core, tile=tile, lane=lane,
                chunk=chunk, sloc=sloc, Jtc=Jtc)


def _build_schedule(Jtc):
    """Static schedule, identical for all cores.

    Returns (groups, total_cols). groups[g] = list of instruction entries
    (chunk, start_col, ncols, [(tile, ncols_t), ...]) covering tiles
    [g*BT, (g+1)*BT); each instruction has ncols <= MAXJ.
    """
    groups = []
    pos = 0
    NG = (NT + BT - 1) // BT
    for g in range(NG):
        t0, t1 = g * BT, min((g + 1) * BT, NT)
        entries = []
        for c in range(NCHUNK):
            cur_tiles, cur_cols, cur_start = [], 0, pos
            for t in range(t0, t1):
                nct = int(Jtc[t, c])
                if nct == 0:
                    continue
                while nct > 0:
                    take = min(nct, MAXJ - cur_cols)
                    if take == 0:
                        entries.append((c, cur_start, cur_cols, cur_tiles))
                        cur_tiles, cur_cols, cur_start = [], 0, pos
                        continue
                    cur_tiles.append((t, take))
                    cur_cols += take
                    pos += take
                    nct -= take
            if cur_cols > 0:
                entries.append((c, cur_start, cur_cols, cur_tiles))
        groups.append(entries)
    return groups, pos


def _build_core_arrays(ep, groups, total_cols, core):
    sel = ep['dst_core'] == core
    tiles = ep['tile'][sel]
    lanes = ep['lane'][sel]
    chunks = ep['chunk'][sel]
    slocs = ep['sloc'][sel]

    key = tiles * NCHUNK + chunks
    order = np.argsort(key, kind='stable')
    lanes, slocs = lanes[order], slocs[order]
    bounds = np.searchsorted(key[order], np.arange(NT * NCHUNK + 1))
    used = np.zeros(NT * NCHUNK, np.int64)         # edges consumed per (tile, chunk)

    idx_flat = np.zeros(total_cols * 128, np.int32)
    dst_flat = np.zeros(total_cols * 128, np.int32)
    for entries in groups:
        for (c, start, ncols, inst_tiles) in entries:
            p = start
            for (t, nct) in inst_tiles:
                bk = t * NCHUNK + c
                b0 = bounds[bk] + used[bk]
                avail = bounds[bk + 1] - b0
                n = min(avail, nct * 128)
                used[bk] += n
                base = p * 128
                idx_flat[base:base + n] = slocs[b0:b0 + n]
                dst_flat[base:base + n] = lanes[b0:b0 + n]
                zloc = ZROW[c] - c * CHUNK
                idx_flat[base + n: base + nct * 128] = zloc
                dst_flat[base + n: base + nct * 128] = 0
                p += nct

    iw = idx_flat.reshape(total_cols * 8, 16)
    idx_wrapped = np.tile(np.ascontiguousarray(iw.T.astype(np.int16)), (8, 1))
    dstloc = np.ascontiguousarray(dst_flat.reshape(total_cols, 128).T.astype(np.float32))
    return idx_wrapped, dstloc


# -------------------------------------------------------------- device build

def _build_nc(groups, total_cols, debug=False):
    import concourse.bacc as bacc
    import concourse.bass as bass
    import concourse.mybir as mybir
    import concourse.tile as tile
    from concourse.masks import make_identity

    f32 = mybir.dt.float32
    AF = mybir.ActivationFunctionType
    OP = mybir.AluOpType
    RG = [list(range(NCORES))]

    nc = bacc.Bacc("TRN2", target_bir_lowering=False, debug=False, num_devices=NCORES)

    # ---- I/O ----
    nfT = nc.dram_tensor('nfT', [128, PCP], f32, kind='ExternalInput')
    degpt = nc.dram_tensor('degpt', [128, NT], f32, kind='ExternalInput')
    idxw = nc.dram_tensor('idxw', [128, total_cols * 8], mybir.dt.int16, kind='ExternalInput')
    dstl = nc.dram_tensor('dstl', [128, total_cols], f32, kind='ExternalInput')
    iot = nc.dram_tensor('iot', [128, 128], f32, kind='ExternalInput')
    ones_r = nc.dram_tensor('ones_r', [1, 128], f32, kind='ExternalInput')
    ones_c = nc.dram_tensor('ones_c', [128, 1], f32, kind='ExternalInput')
    w1 = nc.dram_tensor('w1', [128, 128], f32, kind='ExternalInput')
    w2 = nc.dram_tensor('w2', [128, 128], f32, kind='ExternalInput')
    bn_g = nc.dram_tensor('bn_g', [128, 1], f32, kind='ExternalInput')
    bn_b = nc.dram_tensor('bn_b', [128, 1], f32, kind='ExternalInput')
    b1c = nc.dram_tensor('b1c', [128, 1], f32, kind='ExternalInput')
    b2c = nc.dram_tensor('b2c', [128, 1], f32, kind='ExternalInput')
    cba = nc.dram_tensor('cba', [32, 6 * 128], f32, kind='ExternalInput')     # cb rows
    cbta = nc.dram_tensor('cbta', [128, 6 * 32], f32, kind='ExternalInput')   # cb^T cols
    linw = nc.dram_tensor('linw', [128, OUT_C], f32, kind='ExternalInput')
    linb = nc.dram_tensor('linb', [OUT_C, 1], f32, kind='ExternalInput')
    gnnw = nc.dram_tensor('gnnw', [128, 6], f32, kind='ExternalInput')
    gnnb = nc.dram_tensor('gnnb', [6, 1], f32, kind='ExternalInput')
    zeros44 = nc.dram_tensor('zeros44', [44, 128], f32, kind='ExternalInput')

    out1T = nc.dram_tensor('out1T', [OUT_C, PCP], f32, kind='ExternalOutput')
    gnnT = nc.dram_tensor('gnnT', [6, PCP], f32, kind='ExternalOutput')
    idsD = nc.dram_tensor('idsD', [128, NT * 6], mybir.dt.int32, kind='ExternalOutput')
    lossD = nc.dram_tensor('lossD', [1, 1], f32, kind='ExternalOutput')
    if debug:
        dbgG = nc.dram_tensor('dbgG', [PCP, 128], f32, kind='ExternalOutput')
        dbgX1 = nc.dram_tensor('dbgX1', [128, PCP], f32, kind='ExternalOutput')

    with tile.TileContext(nc) as tc:
        with tc.tile_pool(name='dram', bufs=1, space='DRAM') as dram, \
             tc.tile_pool(name='const', bufs=1) as cpool, \
             tc.tile_pool(name='big', bufs=1) as bigpool, \
             tc.tile_pool(name='msg', bufs=2) as msgpool, \
             tc.tile_pool(name='selp', bufs=2) as selpool, \
             tc.tile_pool(name='work', bufs=2) as work, \
             tc.tile_pool(name='rvq', bufs=2) as rvq, \
             tc.tile_pool(name='psA', bufs=4, space='PSUM') as psA, \
             tc.tile_pool(name='psB', bufs=1, space='PSUM') as psB, \
             tc.tile_pool(name='psC', bufs=1, space='PSUM') as psC, \
             tc.tile_pool(name='psD', bufs=2, space='PSUM') as psD:

            gloc = dram.tile([PCP, 128], f32)
            gloc2 = dram.tile([PCP, 128], f32)
            gfull1 = dram.tile([R, 128], f32, addr_space="Shared")
            gfull2 = dram.tile([R, 128], f32, addr_space="Shared")
            bn_in = dram.tile([128, 2], f32)
            bn_out = dram.tile([128, 2], f32, addr_space="Shared")
            loss_in = dram.tile([1, 2], f32)
            loss_out = dram.tile([1, 2], f32, addr_space="Shared")

            # ---- constants into SBUF ----
            ident = cpool.tile([128, 128], f32)
            make_identity(nc, ident[:])
            iota_sb = cpool.tile([128, 128], f32)
            nc.sync.dma_start(iota_sb[:], iot[:])
            onesr_sb = cpool.tile([1, 128], f32)
            nc.sync.dma_start(onesr_sb[:], ones_r[:])
            onesc_sb = cpool.tile([128, 1], f32)
            nc.sync.dma_start(onesc_sb[:], ones_c[:])
            w1_sb = cpool.tile([128, 128], f32)
            nc.sync.dma_start(w1_sb[:], w1[:])
            w2_sb = cpool.tile([128, 128], f32)
            nc.sync.dma_start(w2_sb[:], w2[:])
            linw_sb = cpool.tile([128, OUT_C], f32)
            nc.sync.dma_start(linw_sb[:], linw[:])
            linb_sb = cpool.tile([OUT_C, 1], f32)
            nc.sync.dma_start(linb_sb[:], linb[:])
            gnnw_sb = cpool.tile([128, 6], f32)
            nc.sync.dma_start(gnnw_sb[:], gnnw[:])
            gnnb_sb = cpool.tile([6, 1], f32)
            nc.sync.dma_start(gnnb_sb[:], gnnb[:])
            bng_sb = cpool.tile([128, 1], f32)
            nc.sync.dma_start(bng_sb[:], bn_g[:])
            bnb_sb = cpool.tile([128, 1], f32)
            nc.sync.dma_start(bnb_sb[:], bn_b[:])
            b1_sb = cpool.tile([128, 1], f32)
            nc.sync.dma_start(b1_sb[:], b1c[:])
            b2_sb = cpool.tile([128, 1], f32)
            nc.sync.dma_start(b2_sb[:], b2c[:])
            cb_sb = cpool.tile([32, 6 * 128], f32)
            nc.sync.dma_start(cb_sb[:], cba[:])
            cbt_sb = cpool.tile([128, 6 * 32], f32)
            nc.sync.dma_start(cbt_sb[:], cbta[:])
            z44_sb = cpool.tile([44, 128], f32)
            nc.sync.dma_start(z44_sb[:], zeros44[:])

            idx_sb = bigpool.tile([128, total_cols * 8], mybir.dt.int16)
            nc.sync.dma_start(idx_sb[:], idxw[:])
            dstl_sb = bigpool.tile([128, total_cols], f32)
            nc.sync.dma_start(dstl_sb[:], dstl[:])

            # dinv in node layout [128, NT] and row layout [1, PCP]
            degpt_sb = cpool.tile([128, NT], f32)
            nc.sync.dma_start(degpt_sb[:], degpt[:])
            dinv_pt = cpool.tile([128, NT], f32)
            nc.vector.reciprocal(dinv_pt[:], degpt_sb[:])
            nc.scalar.activation(dinv_pt[:], dinv_pt[:], AF.Sqrt)

            # 2*cb^T and |c|^2 replicated
            cbt2_sb = cpool.tile([128, 6 * 32], f32)
            nc.scalar.mul(cbt2_sb[:], cbt_sb[:], 2.0)
            cbsq = cpool.tile([32, 6 * 128], f32)
            nc.scalar.activation(cbsq[:], cb_sb[:], AF.Square)
            c2 = cpool.tile([32, 6], f32)
            for k in range(6):
                nc.vector.reduce_sum(c2[:, k:k + 1], cbsq[:, k * 128:(k + 1) * 128],
                                     axis=mybir.AxisListType.X)
            c2T_ps = psD.tile([128, 128], f32, space="PSUM", tag='rv')
            nc.tensor.transpose(c2T_ps[:6, :32], c2[:], ident[:32, :32])
            c2T = cpool.tile([6, 32], f32)
            nc.vector.tensor_copy(c2T[:], c2T_ps[:6, :32])
            # flatten [6, 32] (partition-major) into one row [1, 192] via DMA,
            # then broadcast down all 128 partitions with a k=1 matmul
            c2row = cpool.tile([1, 6 * 32], f32)
            r0 = c2row[:]
            nc.sync.dma_start(
                bass.AP(r0.tensor, r0.offset, [r0.ap[0], [32, 6], [1, 32]]),
                c2T[:])
            c2ps = psB.tile([128, 512], f32, space="PSUM", tag='ps')
            nc.tensor.matmul(c2ps[:, :192], lhsT=onesr_sb[:], rhs=c2row[:],
                             start=True, stop=True)
            c2rep = cpool.tile([128, 6 * 32], f32)
            nc.vector.tensor_copy(c2rep[:], c2ps[:, :192])

            xT = bigpool.tile([128, PCP], f32)       # activations ^T (reused for layer 2)
            bnacc = cpool.tile([128, NT], f32)
            bn2acc = cpool.tile([128, NT], f32)
            lossacc = cpool.tile([128, 2 * NT], f32)
            ids_sb = bigpool.tile([128, NT, 6], mybir.dt.int32)

            def gcn_phase1(lay, rhs_tiles):
                """h^T = W^T x^T ; g = h * dinv (node rows); write gloc; AllGather."""
                W = w1_sb if lay == 0 else w2_sb
                gloc_l = gloc if lay == 0 else gloc2
                gfull_l = gfull1 if lay == 0 else gfull2
                for c0 in range(0, PCP, 512):
                    n = min(512, PCP - c0)
                    ps = psB.tile([128, 512], f32, space="PSUM", tag='ps')
                    nc.tensor.matmul(ps[:, :n], lhsT=W[:], rhs=rhs_tiles(c0, n),
                                     start=True, stop=True)
                    hT = work.tile([128, 512], f32, tag='hT')
                    nc.vector.tensor_copy(hT[:, :n], ps[:, :n])
                    for j in range(n // 128):
                        t = c0 // 128 + j
                        tp = psC.tile([128, 128], f32, space="PSUM", tag='tp')
                        nc.tensor.transpose(tp[:], hT[:, j * 128:(j + 1) * 128], ident[:])
                        gsb = work.tile([128, 128], f32, tag='gsb')
                        nc.vector.tensor_scalar(
                            out=gsb[:], in0=tp[:], scalar1=dinv_pt[:, t:t + 1],
                            scalar2=None, op0=OP.mult)
                        nc.sync.dma_start(gloc_l[t * 128:(t + 1) * 128, :], gsb[:])
                # zero pad rows then AllGather
                nc.sync.dma_start(gloc_l[PC:PCP, :], z44_sb[:])
                nc.gpsimd.collective_compute(
                    "AllGather", OP.bypass, replica_groups=RG,
                    ins=[gloc_l[:]], outs=[gfull_l[:]])

            def gcn_phase2(lay, out_xT):
                gfull_l = gfull1 if lay == 0 else gfull2
                # aggregation, group by group
                for g, entries in enumerate(groups):
                    t0 = g * BT
                    ntg = min(BT, NT - t0)
                    psum_map = {}
                    done = {}
                    tile_total = {}
                    for (c, start, ncols, inst_tiles) in entries:
                        for (t, nct) in inst_tiles:
                            tile_total[t] = tile_total.get(t, 0) + nct
                    for (c, start, ncols, inst_tiles) in entries:
                        M = msgpool.tile([128, MAXJ, 128], f32, tag='M')
                        nidx = ncols * 128
                        nc.gpsimd.dma_gather(
                            M[:, :ncols, :],
                            gfull_l[c * CHUNK:min(R, (c + 1) * CHUNK), :],
                            idx_sb[:, start * 8:(start + ncols) * 8], nidx, nidx, 128)
                        Sel = selpool.tile([128, MAXJ, 128], f32, tag='Sel')
                        i0 = iota_sb[:]
                        nc.vector.tensor_tensor(
                            out=Sel[:, :ncols, :],
                            in0=bass.AP(i0.tensor, i0.offset,
                                        [i0.ap[0], [0, ncols], i0.ap[1]]),
                            in1=dstl_sb[:, start:start + ncols].to_broadcast(
                                [128, ncols, 128]),
                            op=OP.is_equal)
                        p = 0
                        for (t, nct) in inst_tiles:
                            if t not in psum_map:
                                psum_map[t] = psA.tile([128, 128], f32,
                                                       space="PSUM", tag='agg',
                                                       name=f'aggps{t}')
                            for j in range(nct):
                                jj = p + j
                                d = done.get(t, 0)
                                nc.tensor.matmul(
                                    psum_map[t][:],
                                    lhsT=M[:, jj, :], rhs=Sel[:, jj, :],
                                    start=(d == 0), stop=(d == tile_total[t] - 1),
                                    skip_group_check=True)
                                done[t] = d + 1
                            p += nct
                    # evictions: x^T tile = quad slice * dinv (broadcast over rows)
                    # dinv rows for this group: transpose [128, ntg] -> [ntg, 128],
                    # flatten partitions to one row via DMA, broadcast via k=1 matmul
                    dtp = psC.tile([128, 128], f32, space="PSUM", tag='tp')
                    nc.tensor.transpose(dtp[:ntg, :], dinv_pt[:, t0:t0 + ntg], ident[:])
                    dts = work.tile([BT, 128], f32, tag='dts')
                    nc.vector.tensor_copy(dts[:ntg, :], dtp[:ntg, :])
                    drow = work.tile([1, 512], f32, tag='drow')
                    dr = drow[:]
                    nc.sync.dma_start(
                        bass.AP(dr.tensor, dr.offset, [dr.ap[0], [128, ntg], [1, 128]]),
                        dts[:ntg, :])
                    dvp = psB.tile([128, 512], f32, space="PSUM", tag='ps')
                    nc.tensor.matmul(dvp[:, :ntg * 128], lhsT=onesr_sb[:],
                                     rhs=drow[0:1, :ntg * 128],
                                     start=True, stop=True)
                    dvs = work.tile([128, 512], f32, tag='dvs')
                    nc.vector.tensor_copy(dvs[:, :ntg * 128], dvp[:, :ntg * 128])
                    for t in range(t0, t0 + ntg):
                        off = (t - t0) * 128
                        nc.vector.tensor_tensor(
                            out=out_xT[:, t * 128:(t + 1) * 128],
                            in0=psum_map[t][:],
                            in1=dvs[:, off:off + 128], op=OP.mult)

            def rvq_layer(lay, srcT):
                """RVQ over srcT tiles; fills ids_sb cols lay*3.. and lossacc."""
                for t in range(NT):
                    resid = rvq.tile([128, 128], f32, tag='resid')
                    nc.vector.tensor_copy(resid[:], srcT[:, t * 128:(t + 1) * 128])
                    for gr in range(GROUPS):
                        k = lay * 3 + gr
                        sp = psD.tile([128, 128], f32, space="PSUM", tag='rv')
                        nc.tensor.matmul(sp[:, :32], lhsT=resid[:],
                                         rhs=cbt2_sb[:, k * 32:(k + 1) * 32],
                                         start=True, stop=True)
                        s_sb = rvq.tile([128, 32], f32, tag='s_sb')
                        nc.vector.tensor_tensor(out=s_sb[:], in0=sp[:, :32],
                                                in1=c2rep[:, k * 32:(k + 1) * 32],
                                                op=OP.subtract)
                        mx8 = rvq.tile([128, 8], f32, tag='mx8')
                        mi8 = rvq.tile([128, 8], mybir.dt.uint32, tag='mi8')
                        nc.vector.max_with_indices(mx8[:], mi8[:], s_sb[:])
                        nc.vector.tensor_copy(ids_sb[:, t, k:k + 1], mi8[:, 0:1])
                        onehot = rvq.tile([128, 32], f32, tag='onehot')
                        nc.vector.tensor_scalar(
                            out=onehot[:], in0=s_sb[:], scalar1=mx8[:, 0:1],
                            scalar2=None, op0=OP.is_equal)
                        op_ps = psD.tile([128, 128], f32, space="PSUM", tag='rv')
                        nc.tensor.transpose(op_ps[:32, :], onehot[:], ident[:])
                        ohT = rvq.tile([32, 128], f32, tag='ohTs')
                        nc.vector.tensor_copy(ohT[:], op_ps[:32, :])
                        q_ps = psD.tile([128, 128], f32, space="PSUM", tag='rv')
                        nc.tensor.matmul(q_ps[:], lhsT=cb_sb[:, k * 128:(k + 1) * 128],
                                         rhs=ohT[:], start=True, stop=True)
                        nc.vector.tensor_tensor(out=resid[:], in0=resid[:], in1=q_ps[:],
                                                op=OP.subtract)
                    # loss partial for this tile (mask pad nodes on last tile)
                    ncol = 84 if t == NT - 1 else 128
                    sq = rvq.tile([128, 128], f32, tag='sq')
                    nc.scalar.activation(sq[:, :ncol], resid[:, :ncol], AF.Square)
                    nc.vector.reduce_sum(lossacc[:, lay * NT + t: lay * NT + t + 1],
                                         sq[:, :ncol], axis=mybir.AxisListType.X)

            # =================== layer 1 ===================
            def rhs_l1(c0, n):
                buf = work.tile([128, 512], f32, tag='nf')
                nc.sync.dma_start(buf[:, :n], nfT[:, c0:c0 + n])
                return buf[:, :n]
            gcn_phase1(0, rhs_l1)
            if debug:
                nc.sync.dma_start(dbgG[:], gloc[:])
            gcn_phase2(0, xT)
            if debug:
                nc.sync.dma_start(dbgX1[:], xT[:])

            # BN partials
            for t in range(NT):
                xt = xT[:, t * 128:(t + 1) * 128]
                nc.vector.reduce_sum(bnacc[:, t:t + 1], xt, axis=mybir.AxisListType.X)
                sq = work.tile([128, 128], f32, tag='bnsq')
                nc.scalar.activation(sq[:], xt, AF.Square)
                nc.vector.reduce_sum(bn2acc[:, t:t + 1], sq[:], axis=mybir.AxisListType.X)
            bsum = cpool.tile([128, 2], f32)
            nc.vector.reduce_sum(bsum[:, 0:1], bnacc[:], axis=mybir.AxisListType.X)
            nc.vector.reduce_sum(bsum[:, 1:2], bn2acc[:], axis=mybir.AxisListType.X)
            # fold b1: sum(x+b1) = sum + N*b1 ; sum((x+b1)^2) = sum2 + 2 b1 sum + N b1^2
            adj = cpool.tile([128, 2], f32)
            nc.vector.tensor_scalar(out=adj[:, 0:1], in0=b1_sb[:], scalar1=float(N),
                                    scalar2=None, op0=OP.mult)
            nc.vector.tensor_add(adj[:, 0:1], adj[:, 0:1], bsum[:, 0:1])
            tmp = cpool.tile([128, 1], f32)
            nc.vector.tensor_mul(tmp[:], b1_sb[:], bsum[:, 0:1])
            nc.scalar.mul(tmp[:], tmp[:], 2.0)
            nc.vector.tensor_add(adj[:, 1:2], bsum[:, 1:2], tmp[:])
            nc.scalar.activation(tmp[:], b1_sb[:], AF.Square)
            nc.vector.tensor_scalar(out=tmp[:], in0=tmp[:], scalar1=float(N),
                                    scalar2=None, op0=OP.mult)
            nc.vector.tensor_add(adj[:, 1:2], adj[:, 1:2], tmp[:])
            nc.sync.dma_start(bn_in[:], adj[:])
            nc.gpsimd.collective_compute("AllReduce", OP.add, replica_groups=RG,
                                         ins=[bn_in[:]], outs=[bn_out[:]])
            bnred = cpool.tile([128, 2], f32)
            nc.sync.dma_start(bnred[:], bn_out[:])
            mu = cpool.tile([128, 1], f32)
            nc.scalar.mul(mu[:], bnred[:, 0:1], 1.0 / N)
            var = cpool.tile([128, 1], f32)
            nc.scalar.mul(var[:], bnred[:, 1:2], 1.0 / N)
            musq = cpool.tile([128, 1], f32)
            nc.scalar.activation(musq[:], mu[:], AF.Square)
            nc.vector.tensor_sub(var[:], var[:], musq[:])
            nc.vector.tensor_scalar(out=var[:], in0=var[:], scalar1=BN_EPS,
                                    scalar2=None, op0=OP.add)
            rstd = cpool.tile([128, 1], f32)
            nc.vector.reciprocal(rstd[:], var[:])
            nc.scalar.activation(rstd[:], rstd[:], AF.Sqrt)
            sc = cpool.tile([128, 1], f32)
            nc.vector.tensor_mul(sc[:], bng_sb[:], rstd[:])
            # shift = bn_b + (b1 - mu) * sc
            sh = cpool.tile([128, 1], f32)
            nc.vector.tensor_sub(sh[:], b1_sb[:], mu[:])
            nc.vector.tensor_mul(sh[:], sh[:], sc[:])
            nc.vector.tensor_add(sh[:], sh[:], bnb_sb[:])
            # x = relu(sc * x + sh) in place on xT
            for t in range(NT):
                xt = xT[:, t * 128:(t + 1) * 128]
                nc.scalar.activation(xt, xt, AF.Relu, bias=sh[:], scale=sc[:])

            # ============== layer 2 matmul/allgather, RVQ0, aggregation ==============
            # W2 matmul + g2 write + AllGather first (uses xT), then RVQ0 (reads xT),
            # then aggregation-2 overwrites xT.
            def rhs_l2(c0, n):
                return xT[:, c0:c0 + n]
            gcn_phase1(1, rhs_l2)
            rvq_layer(0, xT)      # reads xT while AllGather runs
            gcn_phase2(1, xT)     # aggregation-2 overwrites xT (WAR after RVQ0)

            # x2 += b2 (column-broadcast add)
            for t in range(NT):
                xt = xT[:, t * 128:(t + 1) * 128]
                nc.vector.tensor_scalar(out=xt, in0=xt, scalar1=b2_sb[:],
                                        scalar2=None, op0=OP.add)
            rvq_layer(1, xT)

            # =================== heads ===================
            for c0 in range(0, PCP, 512):
                n = min(512, PCP - c0)
                ps = psB.tile([128, 512], f32, space="PSUM", tag='ps')
                nc.tensor.matmul(ps[:OUT_C, :n], lhsT=linw_sb[:], rhs=xT[:, c0:c0 + n],
                                 start=True, stop=True)
                ob = work.tile([OUT_C, 512], f32, tag='ob')
                nc.vector.tensor_scalar(out=ob[:, :n], in0=ps[:OUT_C, :n],
                                        scalar1=linb_sb[:], scalar2=None, op0=OP.add)
                nc.sync.dma_start(out1T[:, c0:c0 + n], ob[:, :n])
                ps2 = psB.tile([128, 512], f32, space="PSUM", tag='ps')
                nc.tensor.matmul(ps2[:6, :n], lhsT=gnnw_sb[:], rhs=xT[:, c0:c0 + n],
                                 start=True, stop=True)
                ob2 = work.tile([6, 512], f32, tag='ob2')
                nc.vector.tensor_scalar(out=ob2[:, :n], in0=ps2[:6, :n],
                                        scalar1=gnnb_sb[:], scalar2=None, op0=OP.add)
                nc.sync.dma_start(gnnT[:, c0:c0 + n], ob2[:, :n])

            nc.sync.dma_start(idsD[:], ids_sb[:].rearrange("p t c -> p (t c)"))

            # loss: sum lossacc per layer, cross-partition via ones matmul
            lpack = cpool.tile([1, 2], f32)
            for lay in range(2):
                lcol = cpool.tile([128, 1], f32, tag=f'lcol{lay}')
                nc.vector.reduce_sum(lcol[:], lossacc[:, lay * NT:(lay + 1) * NT],
                                     axis=mybir.AxisListType.X)
                lps = psD.tile([128, 128], f32, space="PSUM", tag='rv')
                nc.tensor.matmul(lps[:1, :1], lhsT=onesc_sb[:], rhs=lcol[:],
                                 start=True, stop=True)
                nc.vector.tensor_copy(lpack[:, lay:lay + 1], lps[:1, :1])
            nc.sync.dma_start(loss_in[:], lpack[:])
            nc.gpsimd.collective_compute("AllReduce", OP.add, replica_groups=RG,
                                         ins=[loss_in[:]], outs=[loss_out[:]])
            lred = cpool.tile([1, 2], f32)
            nc.sync.dma_start(lred[:], loss_out[:])
            lfin = cpool.tile([1, 1], f32)
            nc.vector.reduce_sum(lfin[:], lred[:], axis=mybir.AxisListType.X)
            nc.scalar.mul(lfin[:], lfin[:], 1.0 / (N * 128))
            nc.sync.dma_start(lossD[:], lfin[:])

    nc.compile()
    return nc


_CACHE = {}


def kernel(node_feat, edge_index, W1, b1, W2, b2, bn_gamma, bn_beta,
           codebooks, lin_W, lin_b, gnn_W, gnn_b):
    from concourse.bass_utils import run_bass_kernel_spmd

    node_feat = np.asarray(node_feat, np.float32)
    edge_index = np.asarray(edge_index, np.int32)

    ep = _prep_edges(edge_index)
    groups, total_cols = _build_schedule(ep['Jtc'])

    debug = bool(globals().get('DEBUG', False))
    key = ('nc', total_cols, ep['Jtc'].tobytes(), debug)
    if key not in _CACHE:
        _CACHE[key] = _build_nc(groups, total_cols, debug=debug)
    nc = _CACHE[key]

    cb = np.asarray(codebooks, np.float32).reshape(6, NUM_CODES, HID)
    cba = np.ascontiguousarray(cb.transpose(1, 0, 2).reshape(32, 6 * 128))
    cbta = np.ascontiguousarray(cb.transpose(2, 0, 1).reshape(128, 6 * 32))

    shared = {
        'iot': np.tile(np.arange(128, dtype=np.float32), (128, 1)),
        'ones_r': np.ones((1, 128), np.float32),
        'ones_c': np.ones((128, 1), np.float32),
        'w1': np.ascontiguousarray(np.asarray(W1, np.float32)),
        'w2': np.ascontiguousarray(np.asarray(W2, np.float32)),
        'bn_g': np.asarray(bn_gamma, np.float32).reshape(128, 1).copy(),
        'bn_b': np.asarray(bn_beta, np.float32).reshape(128, 1).copy(),
        'b1c': np.asarray(b1, np.float32).reshape(128, 1).copy(),
        'b2c': np.asarray(b2, np.float32).reshape(128, 1).copy(),
        'cba': cba, 'cbta': cbta,
        'linw': np.ascontiguousarray(np.asarray(lin_W, np.float32)),
        'linb': np.asarray(lin_b, np.float32).reshape(OUT_C, 1).copy(),
        'gnnw': np.ascontiguousarray(np.asarray(gnn_W, np.float32)),
        'gnnb': np.asarray(gnn_b, np.float32).reshape(6, 1).copy(),
        'zeros44': np.zeros((44, 128), np.float32),
    }

    deg = ep['deg'].astype(np.float32)
    in_maps = []
    for core in range(NCORES):
        idx_wrapped, dstloc = _build_core_arrays(ep, groups, total_cols, core)
        nf = np.zeros((128, PCP), np.float32)
        nf[:, :PC] = node_feat[core * PC:(core + 1) * PC].T
        dcore = np.ones(PCP, np.float32)
        dcore[:PC] = deg[core * PC:(core + 1) * PC]
        in_maps.append(dict(
            nfT=nf,
            degpt=np.ascontiguousarray(dcore.reshape(NT, 128).T),
            idxw=idx_wrapped, dstl=dstloc,
            **shared))

    import time as _time
    _t0 = _time.time()
    res = run_bass_kernel_spmd(nc, in_maps, core_ids=list(range(NCORES)))
    globals()['LAST_EXEC_S'] = _time.time() - _t0
    if bool(globals().get('PROFILE', False)):
        _t0 = _time.time()
        res = run_bass_kernel_spmd(nc, in_maps, core_ids=list(range(NCORES)))
        globals()['LAST_EXEC_S'] = _time.time() - _t0

    out1 = np.empty((N, OUT_C), np.float32)
    gnn = np.empty((N, 6), np.float32)
    ids = np.empty((N, 6), np.int32)
    for core in range(NCORES):
        r = res.results[core]
        out1[core * PC:(core + 1) * PC] = r['out1T'].T[:PC]
        gnn[core * PC:(core + 1) * PC] = r['gnnT'].T[:PC]
        ids[core * PC:(core + 1) * PC] = (
            r['idsD'].reshape(128, NT, 6).transpose(1, 0, 2).reshape(PCP, 6)[:PC])
    loss = np.float32(res.results[0]['lossD'][0, 0])
    if bool(globals().get('DEBUG', False)):
        globals()['DEBUG_RES'] = res.results
    return out1, loss, ids, gnn


# revision 15
# speedup vs baseline: 1.0163x; 1.0163x over previous
"""Trainium2 Bass kernel for nn_GCN_13657996001664 (2-layer GCN + BN/ReLU + RVQ + heads).

Strategy (8 NeuronCores, node-parallel):
  - Each core owns 12500 nodes (padded to 12544 = 98*128).
  - Per GCN layer: h = x @ W on own nodes; g = h * dinv written to a local DRAM
    block; AllGather replicates the full g table (100352 rows) on every core.
  - Edge aggregation: edges bucketed by (dst tile, src-table chunk); messages
    fetched with dma_gather (int16 indices, 4 chunks of 32768 rows); per
    128-edge column a one-hot selection matrix (built on DVE from a dst-lane
    vector) routes messages into a per-tile PSUM accumulator via the
    TensorEngine: x^T[hid, dst] += msg^T @ Sel.
  - BatchNorm via AllReduce of per-channel partial sums; RVQ argmin via
    s = 2 r.c - |c|^2 and max_with_indices; q via one-hot matmul.
All floating-point math runs on device; the host only shards, reorders
integer index structures, and reassembles outputs.
"""

import numpy as np

N, E, IN_C, HID, OUT_C = 100000, 1600000, 128, 128, 40
NUM_CODES, GROUPS, NUM_LAYERS = 32, 3, 2
BN_EPS = 1e-5
NCORES = 8
PC = N // NCORES              # 12500 own nodes
PCP = 12544                   # padded (98 tiles of 128)
NT = PCP // 128               # 98
R = NCORES * PCP              # 100352 table rows
CHUNK = 32768
NCHUNK = 4
BT = 4                        # tiles per psum quad / gather group
MAXJ = 8                      # max 128-edge columns per gather instruction
ZROW = {0: 12500, 1: 37588, 2: 75220, 3: 100308}   # a zero (pad) row inside each chunk


# ----------------------------------------------------------------- host prep

def _prep_edges(edge_index):
    row = edge_index[0].astype(np.int64)
    col = edge_index[1].astype(np.int64)
    loop = np.arange(N, dtype=np.int64)
    r_all = np.concatenate([row, loop])
    c_all = np.concatenate([col, loop])
    deg = np.bincount(r_all, minlength=N)          # includes self-loop

    owner = c_all // PC
    slot = owner * PCP + (c_all - owner * PC)
    chunk = slot // CHUNK
    sloc = slot - chunk * CHUNK

    dst_core = r_all // PC
    dst_local = r_all - dst_core * PC
    tile = dst_local // 128
    lane = dst_local % 128

    counts = np.zeros((NCORES, NT, NCHUNK), np.int64)
    np.add.at(counts, (dst_core, tile, chunk), 1)
    Jtc = (counts.max(axis=0) + 127) // 128        # [NT, NCHUNK] cols, shared by all cores
    return dict(deg=deg, dst_core=dst_core, tile=tile, lane=lane,
                chunk=chunk, sloc=sloc, Jtc=Jtc)


def _build_schedule(Jtc):
    """Static schedule, identical for all cores.

    Returns (groups, total_cols). groups[g] = list of instruction entries
    (chunk, start_col, ncols, [(tile, ncols_t), ...]) covering tiles
    [g*BT, (g+1)*BT); each instruction has ncols <= MAXJ.
    """
    groups = []
    pos = 0
    NG = (NT + BT - 1) // BT
    for g in range(NG):
        t0, t1 = g * BT, min((g + 1) * BT, NT)
        entries = []
        for c in range(NCHUNK):
            cur_tiles, cur_cols, cur_start = [], 0, pos
            for t in range(t0, t1):
                nct = int(Jtc[t, c])
                if nct == 0:
                    continue
                while nct > 0:
                    take = min(nct, MAXJ - cur_cols)
                    if take == 0:
                        entries.append((c, cur_start, cur_cols, cur_tiles))
                        cur_tiles, cur_cols, cur_start = [], 0, pos
                        continue
                    cur_tiles.append((t, take))
                    cur_cols += take
                    pos += take
                    nct -= take
            if cur_cols > 0:
                entries.append((c, cur_start, cur_cols, cur_tiles))
        groups.append(entries)
    return groups, pos


def _build_core_arrays(ep, groups, total_cols, core):
    sel = ep['dst_core'] == core
    tiles = ep['tile'][sel]
    lanes = ep['lane'][sel]
    chunks = ep['chunk'][sel]
    slocs = ep['sloc'][sel]

    key = tiles * NCHUNK + chunks
    order = np.argsort(key, kind='stable')
    lanes, slocs = lanes[order], slocs[order]
    bounds = np.searchsorted(key[order], np.arange(NT * NCHUNK + 1))
    used = np.zeros(NT * NCHUNK, np.int64)         # edges consumed per (tile, chunk)

    idx_flat = np.zeros(total_cols * 128, np.int32)
    dst_flat = np.zeros(total_cols * 128, np.int32)
    for entries in groups:
        for (c, start, ncols, inst_tiles) in entries:
            p = start
            for (t, nct) in inst_tiles:
                bk = t * NCHUNK + c
                b0 = bounds[bk] + used[bk]
                avail = bounds[bk + 1] - b0
                n = min(avail, nct * 128)
                used[bk] += n
                base = p * 128
                idx_flat[base:base + n] = slocs[b0:b0 + n]
                dst_flat[base:base + n] = lanes[b0:b0 + n]
                zloc = ZROW[c] - c * CHUNK
                idx_flat[base + n: base + nct * 128] = zloc
                dst_flat[base + n: base + nct * 128] = 0
                p += nct

    iw = idx_flat.reshape(total_cols * 8, 16)
    idx_wrapped = np.tile(np.ascontiguousarray(iw.T.astype(np.int16)), (8, 1))
    dstloc = np.ascontiguousarray(dst_flat.reshape(total_cols, 128).T.astype(np.float32))
    return idx_wrapped, dstloc


# -------------------------------------------------------------- device build

def _build_nc(groups, total_cols, debug=False):
    import concourse.bacc as bacc
    import concourse.bass as bass
    import concourse.mybir as mybir
    import concourse.tile as tile
    from concourse.masks import make_identity

    f32 = mybir.dt.float32
    AF = mybir.ActivationFunctionType
    OP = mybir.AluOpType
    RG = [list(range(NCORES))]

    nc = bacc.Bacc("TRN2", target_bir_lowering=False, debug=False, num_devices=NCORES)

    # ---- I/O ----
    nfT = nc.dram_tensor('nfT', [128, PCP], f32, kind='ExternalInput')
    degpt = nc.dram_tensor('degpt', [128, NT], f32, kind='ExternalInput')
    idxw = nc.dram_tensor('idxw', [128, total_cols * 8], mybir.dt.int16, kind='ExternalInput')
    dstl = nc.dram_tensor('dstl', [128, total_cols], f32, kind='ExternalInput')
    iot = nc.dram_tensor('iot', [128, 128], f32, kind='ExternalInput')
    ones_r = nc.dram_tensor('ones_r', [1, 128], f32, kind='ExternalInput')
    ones_c = nc.dram_tensor('ones_c', [128, 1], f32, kind='ExternalInput')
    w1 = nc.dram_tensor('w1', [128, 128], f32, kind='ExternalInput')
    w2 = nc.dram_tensor('w2', [128, 128], f32, kind='ExternalInput')
    bn_g = nc.dram_tensor('bn_g', [128, 1], f32, kind='ExternalInput')
    bn_b = nc.dram_tensor('bn_b', [128, 1], f32, kind='ExternalInput')
    b1c = nc.dram_tensor('b1c', [128, 1], f32, kind='ExternalInput')
    b2c = nc.dram_tensor('b2c', [128, 1], f32, kind='ExternalInput')
    cba = nc.dram_tensor('cba', [32, 6 * 128], f32, kind='ExternalInput')     # cb rows
    cbta = nc.dram_tensor('cbta', [128, 6 * 32], f32, kind='ExternalInput')   # cb^T cols
    linw = nc.dram_tensor('linw', [128, OUT_C], f32, kind='ExternalInput')
    linb = nc.dram_tensor('linb', [OUT_C, 1], f32, kind='ExternalInput')
    gnnw = nc.dram_tensor('gnnw', [128, 6], f32, kind='ExternalInput')
    gnnb = nc.dram_tensor('gnnb', [6, 1], f32, kind='ExternalInput')
    zeros44 = nc.dram_tensor('zeros44', [44, 128], f32, kind='ExternalInput')

    out1T = nc.dram_tensor('out1T', [OUT_C, PCP], f32, kind='ExternalOutput')
    gnnT = nc.dram_tensor('gnnT', [6, PCP], f32, kind='ExternalOutput')
    idsD = nc.dram_tensor('idsD', [128, NT * 6], mybir.dt.int32, kind='ExternalOutput')
    lossD = nc.dram_tensor('lossD', [1, 1], f32, kind='ExternalOutput')
    if debug:
        dbgG = nc.dram_tensor('dbgG', [PCP, 128], f32, kind='ExternalOutput')
        dbgX1 = nc.dram_tensor('dbgX1', [128, PCP], f32, kind='ExternalOutput')

    with tile.TileContext(nc) as tc:
        with tc.tile_pool(name='dram', bufs=1, space='DRAM') as dram, \
             tc.tile_pool(name='const', bufs=1) as cpool, \
             tc.tile_pool(name='big', bufs=1) as bigpool, \
             tc.tile_pool(name='msg', bufs=3) as msgpool, \
             tc.tile_pool(name='selp', bufs=3) as selpool, \
             tc.tile_pool(name='work', bufs=2) as work, \
             tc.tile_pool(name='rvq', bufs=2) as rvq, \
             tc.tile_pool(name='psA', bufs=4, space='PSUM') as psA, \
             tc.tile_pool(name='psB', bufs=1, space='PSUM') as psB, \
             tc.tile_pool(name='psC', bufs=1, space='PSUM') as psC, \
             tc.tile_pool(name='psD', bufs=2, space='PSUM') as psD:

            gloc = dram.tile([PCP, 128], f32)
            gloc2 = dram.tile([PCP, 128], f32)
            gfull1 = dram.tile([R, 128], f32, addr_space="Shared")
            gfull2 = dram.tile([R, 128], f32, addr_space="Shared")
            bn_in = dram.tile([128, 2], f32)
            bn_out = dram.tile([128, 2], f32, addr_space="Shared")
            loss_in = dram.tile([1, 2], f32)
            loss_out = dram.tile([1, 2], f32, addr_space="Shared")

            # ---- constants into SBUF ----
            ident = cpool.tile([128, 128], f32)
            make_identity(nc, ident[:])
            iota_sb = cpool.tile([128, 128], f32)
            nc.sync.dma_start(iota_sb[:], iot[:])
            onesr_sb = cpool.tile([1, 128], f32)
            nc.sync.dma_start(onesr_sb[:], ones_r[:])
            onesc_sb = cpool.tile([128, 1], f32)
            nc.sync.dma_start(onesc_sb[:], ones_c[:])
            w1_sb = cpool.tile([128, 128], f32)
            nc.sync.dma_start(w1_sb[:], w1[:])
            w2_sb = cpool.tile([128, 128], f32)
            nc.sync.dma_start(w2_sb[:], w2[:])
            linw_sb = cpool.tile([128, OUT_C], f32)
            nc.sync.dma_start(linw_sb[:], linw[:])
            linb_sb = cpool.tile([OUT_C, 1], f32)
            nc.sync.dma_start(linb_sb[:], linb[:])
            gnnw_sb = cpool.tile([128, 6], f32)
            nc.sync.dma_start(gnnw_sb[:], gnnw[:])
            gnnb_sb = cpool.tile([6, 1], f32)
            nc.sync.dma_start(gnnb_sb[:], gnnb[:])
            bng_sb = cpool.tile([128, 1], f32)
            nc.sync.dma_start(bng_sb[:], bn_g[:])
            bnb_sb = cpool.tile([128, 1], f32)
            nc.sync.dma_start(bnb_sb[:], bn_b[:])
            b1_sb = cpool.tile([128, 1], f32)
            nc.sync.dma_start(b1_sb[:], b1c[:])
            b2_sb = cpool.tile([128, 1], f32)
            nc.sync.dma_start(b2_sb[:], b2c[:])
            cb_sb = cpool.tile([32, 6 * 128], f32)
            nc.sync.dma_start(cb_sb[:], cba[:])
            cbt_sb = cpool.tile([128, 6 * 32], f32)
            nc.sync.dma_start(cbt_sb[:], cbta[:])
            z44_sb = cpool.tile([44, 128], f32)
            nc.sync.dma_start(z44_sb[:], zeros44[:])

            idx_sb = bigpool.tile([128, total_cols * 8], mybir.dt.int16)
            nc.sync.dma_start(idx_sb[:], idxw[:])
            dstl_sb = bigpool.tile([128, total_cols], f32)
            nc.sync.dma_start(dstl_sb[:], dstl[:])

            # dinv in node layout [128, NT] and row layout [1, PCP]
            degpt_sb = cpool.tile([128, NT], f32)
            nc.sync.dma_start(degpt_sb[:], degpt[:])
            dinv_pt = cpool.tile([128, NT], f32)
            nc.vector.reciprocal(dinv_pt[:], degpt_sb[:])
            nc.scalar.activation(dinv_pt[:], dinv_pt[:], AF.Sqrt)

            # 2*cb^T and |c|^2 replicated
            cbt2_sb = cpool.tile([128, 6 * 32], f32)
            nc.scalar.mul(cbt2_sb[:], cbt_sb[:], 2.0)
            cbsq = cpool.tile([32, 6 * 128], f32)
            nc.scalar.activation(cbsq[:], cb_sb[:], AF.Square)
            c2 = cpool.tile([32, 6], f32)
            for k in range(6):
                nc.vector.reduce_sum(c2[:, k:k + 1], cbsq[:, k * 128:(k + 1) * 128],
                                     axis=mybir.AxisListType.X)
            c2T_ps = psD.tile([128, 128], f32, space="PSUM", tag='rv')
            nc.tensor.transpose(c2T_ps[:6, :32], c2[:], ident[:32, :32])
            c2T = cpool.tile([6, 32], f32)
            nc.vector.tensor_copy(c2T[:], c2T_ps[:6, :32])
            # flatten [6, 32] (partition-major) into one row [1, 192] via DMA,
            # then broadcast down all 128 partitions with a k=1 matmul
            c2row = cpool.tile([1, 6 * 32], f32)
            r0 = c2row[:]
            nc.sync.dma_start(
                bass.AP(r0.tensor, r0.offset, [r0.ap[0], [32, 6], [1, 32]]),
                c2T[:])
            c2ps = psB.tile([128, 512], f32, space="PSUM", tag='ps')
            nc.tensor.matmul(c2ps[:, :192], lhsT=onesr_sb[:], rhs=c2row[:],
                             start=True, stop=True)
            c2rep = cpool.tile([128, 6 * 32], f32)
            nc.vector.tensor_copy(c2rep[:], c2ps[:, :192])

            xT = bigpool.tile([128, PCP], f32)       # activations ^T (reused for layer 2)
            bnacc = cpool.tile([128, NT], f32)
            bn2acc = cpool.tile([128, NT], f32)
            lossacc = cpool.tile([128, 2 * NT], f32)
            ids_sb = bigpool.tile([128, NT, 6], mybir.dt.int32)

            def gcn_phase1(lay, rhs_tiles):
                """h^T = W^T x^T ; g = h * dinv (node rows); write gloc; AllGather."""
                W = w1_sb if lay == 0 else w2_sb
                gloc_l = gloc if lay == 0 else gloc2
                gfull_l = gfull1 if lay == 0 else gfull2
                for c0 in range(0, PCP, 512):
                    n = min(512, PCP - c0)
                    ps = psB.tile([128, 512], f32, space="PSUM", tag='ps')
                    nc.tensor.matmul(ps[:, :n], lhsT=W[:], rhs=rhs_tiles(c0, n),
                                     start=True, stop=True)
                    hT = work.tile([128, 512], f32, tag='hT')
                    nc.vector.tensor_copy(hT[:, :n], ps[:, :n])
                    for j in range(n // 128):
                        t = c0 // 128 + j
                        tp = psC.tile([128, 128], f32, space="PSUM", tag='tp')
                        nc.tensor.transpose(tp[:], hT[:, j * 128:(j + 1) * 128], ident[:])
                        gsb = work.tile([128, 128], f32, tag='gsb')
                        nc.vector.tensor_scalar(
                            out=gsb[:], in0=tp[:], scalar1=dinv_pt[:, t:t + 1],
                            scalar2=None, op0=OP.mult)
                        nc.sync.dma_start(gloc_l[t * 128:(t + 1) * 128, :], gsb[:])
                # zero pad rows then AllGather
                nc.sync.dma_start(gloc_l[PC:PCP, :], z44_sb[:])
                nc.gpsimd.collective_compute(
                    "AllGather", OP.bypass, replica_groups=RG,
                    ins=[gloc_l[:]], outs=[gfull_l[:]])

            def gcn_phase2(lay, out_xT):
                gfull_l = gfull1 if lay == 0 else gfull2
                # aggregation, group by group
                for g, entries in enumerate(groups):
                    t0 = g * BT
                    ntg = min(BT, NT - t0)
                    psum_map = {}
                    done = {}
                    tile_total = {}
                    for (c, start, ncols, inst_tiles) in entries:
                        for (t, nct) in inst_tiles:
                            tile_total[t] = tile_total.get(t, 0) + nct
                    for (c, start, ncols, inst_tiles) in entries:
                        M = msgpool.tile([128, MAXJ, 128], f32, tag='M')
                        nidx = ncols * 128
                        nc.gpsimd.dma_gather(
                            M[:, :ncols, :],
                            gfull_l[c * CHUNK:min(R, (c + 1) * CHUNK), :],
                            idx_sb[:, start * 8:(start + ncols) * 8], nidx, nidx, 128)
                        Sel = selpool.tile([128, MAXJ, 128], f32, tag='Sel')
                        i0 = iota_sb[:]
                        nc.vector.tensor_tensor(
                            out=Sel[:, :ncols, :],
                            in0=bass.AP(i0.tensor, i0.offset,
                                        [i0.ap[0], [0, ncols], i0.ap[1]]),
                            in1=dstl_sb[:, start:start + ncols].to_broadcast(
                                [128, ncols, 128]),
                            op=OP.is_equal)
                        p = 0
                        for (t, nct) in inst_tiles:
                            if t not in psum_map:
                                psum_map[t] = psA.tile([128, 128], f32,
                                                       space="PSUM", tag='agg',
                                                       name=f'aggps{t}')
                            for j in range(nct):
                                jj = p + j
                                d = done.get(t, 0)
                                nc.tensor.matmul(
                                    psum_map[t][:],
                                    lhsT=M[:, jj, :], rhs=Sel[:, jj, :],
                                    start=(d == 0), stop=(d == tile_total[t] - 1),
                                    skip_group_check=True)
                                done[t] = d + 1
                            p += nct
                    # evictions: x^T tile = quad slice * dinv (broadcast over rows)
                    # dinv rows for this group: transpose [128, ntg] -> [ntg, 128],
                    # flatten partitions to one row via DMA, broadcast via k=1 matmul
                    dtp = psC.tile([128, 128], f32, space="PSUM", tag='tp')
                    nc.tensor.transpose(dtp[:ntg, :], dinv_pt[:, t0:t0 + ntg], ident[:])
                    dts = work.tile([BT, 128], f32, tag='dts')
                    nc.vector.tensor_copy(dts[:ntg, :], dtp[:ntg, :])
                    drow = work.tile([1, 512], f32, tag='drow')
                    dr = drow[:]
                    nc.sync.dma_start(
                        bass.AP(dr.tensor, dr.offset, [dr.ap[0], [128, ntg], [1, 128]]),
                        dts[:ntg, :])
                    dvp = psB.tile([128, 512], f32, space="PSUM", tag='ps')
                    nc.tensor.matmul(dvp[:, :ntg * 128], lhsT=onesr_sb[:],
                                     rhs=drow[0:1, :ntg * 128],
                                     start=True, stop=True)
                    dvs = work.tile([128, 512], f32, tag='dvs')
                    nc.vector.tensor_copy(dvs[:, :ntg * 128], dvp[:, :ntg * 128])
                    for t in range(t0, t0 + ntg):
                        off = (t - t0) * 128
                        nc.vector.tensor_tensor(
                            out=out_xT[:, t * 128:(t + 1) * 128],
                            in0=psum_map[t][:],
                            in1=dvs[:, off:off + 128], op=OP.mult)

            def rvq_layer(lay, srcT):
                """RVQ over srcT tiles; fills ids_sb cols lay*3.. and lossacc."""
                for t in range(NT):
                    resid = rvq.tile([128, 128], f32, tag='resid')
                    nc.vector.tensor_copy(resid[:], srcT[:, t * 128:(t + 1) * 128])
                    for gr in range(GROUPS):
                        k = lay * 3 + gr
                        sp = psD.tile([128, 128], f32, space="PSUM", tag='rv')
                        nc.tensor.matmul(sp[:, :32], lhsT=resid[:],
                                         rhs=cbt2_sb[:, k * 32:(k + 1) * 32],
                                         start=True, stop=True)
                        s_sb = rvq.tile([128, 32], f32, tag='s_sb')
                        nc.vector.tensor_tensor(out=s_sb[:], in0=sp[:, :32],
                                                in1=c2rep[:, k * 32:(k + 1) * 32],
                                                op=OP.subtract)
                        mx8 = rvq.tile([128, 8], f32, tag='mx8')
                        mi8 = rvq.tile([128, 8], mybir.dt.uint32, tag='mi8')
                        nc.vector.max_with_indices(mx8[:], mi8[:], s_sb[:])
                        nc.vector.tensor_copy(ids_sb[:, t, k:k + 1], mi8[:, 0:1])
                        onehot = rvq.tile([128, 32], f32, tag='onehot')
                        nc.vector.tensor_scalar(
                            out=onehot[:], in0=s_sb[:], scalar1=mx8[:, 0:1],
                            scalar2=None, op0=OP.is_equal)
                        op_ps = psD.tile([128, 128], f32, space="PSUM", tag='rv')
                        nc.tensor.transpose(op_ps[:32, :], onehot[:], ident[:])
                        ohT = rvq.tile([32, 128], f32, tag='ohTs')
                        nc.vector.tensor_copy(ohT[:], op_ps[:32, :])
                        q_ps = psD.tile([128, 128], f32, space="PSUM", tag='rv')
                        nc.tensor.matmul(q_ps[:], lhsT=cb_sb[:, k * 128:(k + 1) * 128],
                                         rhs=ohT[:], start=True, stop=True)
                        nc.vector.tensor_tensor(out=resid[:], in0=resid[:], in1=q_ps[:],
                                                op=OP.subtract)
                    # loss partial for this tile (mask pad nodes on last tile)
                    ncol = 84 if t == NT - 1 else 128
                    sq = rvq.tile([128, 128], f32, tag='sq')
                    nc.scalar.activation(sq[:, :ncol], resid[:, :ncol], AF.Square)
                    nc.vector.reduce_sum(lossacc[:, lay * NT + t: lay * NT + t + 1],
                                         sq[:, :ncol], axis=mybir.AxisListType.X)

            # =================== layer 1 ===================
            def rhs_l1(c0, n):
                buf = work.tile([128, 512], f32, tag='nf')
                nc.sync.dma_start(buf[:, :n], nfT[:, c0:c0 + n])
                return buf[:, :n]
            gcn_phase1(0, rhs_l1)
            if debug:
                nc.sync.dma_start(dbgG[:], gloc[:])
            gcn_phase2(0, xT)
            if debug:
                nc.sync.dma_start(dbgX1[:], xT[:])

            # BN partials
            for t in range(NT):
                xt = xT[:, t * 128:(t + 1) * 128]
                nc.vector.reduce_sum(bnacc[:, t:t + 1], xt, axis=mybir.AxisListType.X)
                sq = work.tile([128, 128], f32, tag='bnsq')
                nc.scalar.activation(sq[:], xt, AF.Square)
                nc.vector.reduce_sum(bn2acc[:, t:t + 1], sq[:], axis=mybir.AxisListType.X)
            bsum = cpool.tile([128, 2], f32)
            nc.vector.reduce_sum(bsum[:, 0:1], bnacc[:], axis=mybir.AxisListType.X)
            nc.vector.reduce_sum(bsum[:, 1:2], bn2acc[:], axis=mybir.AxisListType.X)
            # fold b1: sum(x+b1) = sum + N*b1 ; sum((x+b1)^2) = sum2 + 2 b1 sum + N b1^2
            adj = cpool.tile([128, 2], f32)
            nc.vector.tensor_scalar(out=adj[:, 0:1], in0=b1_sb[:], scalar1=float(N),
                                    scalar2=None, op0=OP.mult)
            nc.vector.tensor_add(adj[:, 0:1], adj[:, 0:1], bsum[:, 0:1])
            tmp = cpool.tile([128, 1], f32)
            nc.vector.tensor_mul(tmp[:], b1_sb[:], bsum[:, 0:1])
            nc.scalar.mul(tmp[:], tmp[:], 2.0)
            nc.vector.tensor_add(adj[:, 1:2], bsum[:, 1:2], tmp[:])
            nc.scalar.activation(tmp[:], b1_sb[:], AF.Square)
            nc.vector.tensor_scalar(out=tmp[:], in0=tmp[:], scalar1=float(N),
                                    scalar2=None, op0=OP.mult)
            nc.vector.tensor_add(adj[:, 1:2], adj[:, 1:2], tmp[:])
            nc.sync.dma_start(bn_in[:], adj[:])
            nc.gpsimd.collective_compute("AllReduce", OP.add, replica_groups=RG,
                                         ins=[bn_in[:]], outs=[bn_out[:]])
            bnred = cpool.tile([128, 2], f32)
            nc.sync.dma_start(bnred[:], bn_out[:])
            mu = cpool.tile([128, 1], f32)
            nc.scalar.mul(mu[:], bnred[:, 0:1], 1.0 / N)
            var = cpool.tile([128, 1], f32)
            nc.scalar.mul(var[:], bnred[:, 1:2], 1.0 / N)
            musq = cpool.tile([128, 1], f32)
            nc.scalar.activation(musq[:], mu[:], AF.Square)
            nc.vector.tensor_sub(var[:], var[:], musq[:])
            nc.vector.tensor_scalar(out=var[:], in0=var[:], scalar1=BN_EPS,
                                    scalar2=None, op0=OP.add)
            rstd = cpool.tile([128, 1], f32)
            nc.vector.reciprocal(rstd[:], var[:])
            nc.scalar.activation(rstd[:], rstd[:], AF.Sqrt)
            sc = cpool.tile([128, 1], f32)
            nc.vector.tensor_mul(sc[:], bng_sb[:], rstd[:])
            # shift = bn_b + (b1 - mu) * sc
            sh = cpool.tile([128, 1], f32)
            nc.vector.tensor_sub(sh[:], b1_sb[:], mu[:])
            nc.vector.tensor_mul(sh[:], sh[:], sc[:])
            nc.vector.tensor_add(sh[:], sh[:], bnb_sb[:])
            # x = relu(sc * x + sh) in place on xT
            for t in range(NT):
                xt = xT[:, t * 128:(t + 1) * 128]
                nc.scalar.activation(xt, xt, AF.Relu, bias=sh[:], scale=sc[:])

            # ============== layer 2 matmul/allgather, RVQ0, aggregation ==============
            # W2 matmul + g2 write + AllGather first (uses xT), then RVQ0 (reads xT),
            # then aggregation-2 overwrites xT.
            def rhs_l2(c0, n):
                return xT[:, c0:c0 + n]
            gcn_phase1(1, rhs_l2)
            rvq_layer(0, xT)      # reads xT while AllGather runs
            gcn_phase2(1, xT)     # aggregation-2 overwrites xT (WAR after RVQ0)

            # x2 += b2 (column-broadcast add)
            for t in range(NT):
                xt = xT[:, t * 128:(t + 1) * 128]
                nc.vector.tensor_scalar(out=xt, in0=xt, scalar1=b2_sb[:],
                                        scalar2=None, op0=OP.add)
            rvq_layer(1, xT)

            # =================== heads ===================
            for c0 in range(0, PCP, 512):
                n = min(512, PCP - c0)
                ps = psB.tile([128, 512], f32, space="PSUM", tag='ps')
                nc.tensor.matmul(ps[:OUT_C, :n], lhsT=linw_sb[:], rhs=xT[:, c0:c0 + n],
                                 start=True, stop=True)
                ob = work.tile([OUT_C, 512], f32, tag='ob')
                nc.vector.tensor_scalar(out=ob[:, :n], in0=ps[:OUT_C, :n],
                                        scalar1=linb_sb[:], scalar2=None, op0=OP.add)
                nc.sync.dma_start(out1T[:, c0:c0 + n], ob[:, :n])
                ps2 = psB.tile([128, 512], f32, space="PSUM", tag='ps')
                nc.tensor.matmul(ps2[:6, :n], lhsT=gnnw_sb[:], rhs=xT[:, c0:c0 + n],
                                 start=True, stop=True)
                ob2 = work.tile([6, 512], f32, tag='ob2')
                nc.vector.tensor_scalar(out=ob2[:, :n], in0=ps2[:6, :n],
                                        scalar1=gnnb_sb[:], scalar2=None, op0=OP.add)
                nc.sync.dma_start(gnnT[:, c0:c0 + n], ob2[:, :n])

            nc.sync.dma_start(idsD[:], ids_sb[:].rearrange("p t c -> p (t c)"))

            # loss: sum lossacc per layer, cross-partition via ones matmul
            lpack = cpool.tile([1, 2], f32)
            for lay in range(2):
                lcol = cpool.tile([128, 1], f32, tag=f'lcol{lay}')
                nc.vector.reduce_sum(lcol[:], lossacc[:, lay * NT:(lay + 1) * NT],
                                     axis=mybir.AxisListType.X)
                lps = psD.tile([128, 128], f32, space="PSUM", tag='rv')
                nc.tensor.matmul(lps[:1, :1], lhsT=onesc_sb[:], rhs=lcol[:],
                                 start=True, stop=True)
                nc.vector.tensor_copy(lpack[:, lay:lay + 1], lps[:1, :1])
            nc.sync.dma_start(loss_in[:], lpack[:])
            nc.gpsimd.collective_compute("AllReduce", OP.add, replica_groups=RG,
                                         ins=[loss_in[:]], outs=[loss_out[:]])
            lred = cpool.tile([1, 2], f32)
            nc.sync.dma_start(lred[:], loss_out[:])
            lfin = cpool.tile([1, 1], f32)
            nc.vector.reduce_sum(lfin[:], lred[:], axis=mybir.AxisListType.X)
            nc.scalar.mul(lfin[:], lfin[:], 1.0 / (N * 128))
            nc.sync.dma_start(lossD[:], lfin[:])

    nc.compile()
    return nc


_CACHE = {}


def kernel(node_feat, edge_index, W1, b1, W2, b2, bn_gamma, bn_beta,
           codebooks, lin_W, lin_b, gnn_W, gnn_b):
    from concourse.bass_utils import run_bass_kernel_spmd

    node_feat = np.asarray(node_feat, np.float32)
    edge_index = np.asarray(edge_index, np.int32)

    ep = _prep_edges(edge_index)
    groups, total_cols = _build_schedule(ep['Jtc'])

    debug = bool(globals().get('DEBUG', False))
    key = ('nc', total_cols, ep['Jtc'].tobytes(), debug)
    if key not in _CACHE:
        _CACHE[key] = _build_nc(groups, total_cols, debug=debug)
    nc = _CACHE[key]

    cb = np.asarray(codebooks, np.float32).reshape(6, NUM_CODES, HID)
    cba = np.ascontiguousarray(cb.transpose(1, 0, 2).reshape(32, 6 * 128))
    cbta = np.ascontiguousarray(cb.transpose(2, 0, 1).reshape(128, 6 * 32))

    shared = {
        'iot': np.tile(np.arange(128, dtype=np.float32), (128, 1)),
        'ones_r': np.ones((1, 128), np.float32),
        'ones_c': np.ones((128, 1), np.float32),
        'w1': np.ascontiguousarray(np.asarray(W1, np.float32)),
        'w2': np.ascontiguousarray(np.asarray(W2, np.float32)),
        'bn_g': np.asarray(bn_gamma, np.float32).reshape(128, 1).copy(),
        'bn_b': np.asarray(bn_beta, np.float32).reshape(128, 1).copy(),
        'b1c': np.asarray(b1, np.float32).reshape(128, 1).copy(),
        'b2c': np.asarray(b2, np.float32).reshape(128, 1).copy(),
        'cba': cba, 'cbta': cbta,
        'linw': np.ascontiguousarray(np.asarray(lin_W, np.float32)),
        'linb': np.asarray(lin_b, np.float32).reshape(OUT_C, 1).copy(),
        'gnnw': np.ascontiguousarray(np.asarray(gnn_W, np.float32)),
        'gnnb': np.asarray(gnn_b, np.float32).reshape(6, 1).copy(),
        'zeros44': np.zeros((44, 128), np.float32),
    }

    deg = ep['deg'].astype(np.float32)
    in_maps = []
    for core in range(NCORES):
        idx_wrapped, dstloc = _build_core_arrays(ep, groups, total_cols, core)
        nf = np.zeros((128, PCP), np.float32)
        nf[:, :PC] = node_feat[core * PC:(core + 1) * PC].T
        dcore = np.ones(PCP, np.float32)
        dcore[:PC] = deg[core * PC:(core + 1) * PC]
        in_maps.append(dict(
            nfT=nf,
            degpt=np.ascontiguousarray(dcore.reshape(NT, 128).T),
            idxw=idx_wrapped, dstl=dstloc,
            **shared))

    import time as _time
    _t0 = _time.time()
    res = run_bass_kernel_spmd(nc, in_maps, core_ids=list(range(NCORES)))
    globals()['LAST_EXEC_S'] = _time.time() - _t0
    if bool(globals().get('PROFILE', False)):
        _t0 = _time.time()
        res = run_bass_kernel_spmd(nc, in_maps, core_ids=list(range(NCORES)))
        globals()['LAST_EXEC_S'] = _time.time() - _t0

    out1 = np.empty((N, OUT_C), np.float32)
    gnn = np.empty((N, 6), np.float32)
    ids = np.empty((N, 6), np.int32)
    for core in range(NCORES):
        r = res.results[core]
        out1[core * PC:(core + 1) * PC] = r['out1T'].T[:PC]
        gnn[core * PC:(core + 1) * PC] = r['gnnT'].T[:PC]
        ids[core * PC:(core + 1) * PC] = (
            r['idsD'].reshape(128, NT, 6).transpose(1, 0, 2).reshape(PCP, 6)[:PC])
    loss = np.float32(res.results[0]['lossD'][0, 0])
    if bool(globals().get('DEBUG', False)):
        globals()['DEBUG_RES'] = res.results
    return out1, loss, ids, gnn
